# revision 1
# baseline (speedup 1.0000x reference)
"""Trainium2 Bass kernel for nn_CombinedCriterionAEImpulse (retrieval_knn).

Computes, on 8 NeuronCores, the heavy part of the loss:
  - q[i, j]      = 2*p_i . g_j - |g_j|^2  over the full (8192, 32768) pred x gt
    grid (row max of q  <=>  row min of squared distance), reduced on-device to
    per-row maxes over groups of 64 columns.
  - qself[i, j]  = 2*p_i . p_j - |p_j|^2  over (8192, 8192) pred x pred with the
    diagonal masked, reduced the same way (groups of 64).
Rows (pred points) are sharded across the 8 cores; each core also emits the
group-level maxima.  The host then resolves the winning 64-wide group per row
(trivial numpy), gathers gt points/normals, and combines the scalar loss terms.
"""

import numpy as np

try:
    import concourse.bass as bass
except ImportError:  # pragma: no cover
    import sys

    sys.path.insert(0, "/opt/trn_rl_repo")
    import concourse.bass as bass

import concourse.mybir as mybir
import concourse.tile as tile
from concourse import bacc
from concourse.bass_utils import run_bass_kernel_spmd

P = 128
F32 = mybir.dt.float32
BF16 = mybir.dt.bfloat16
K = 11

NPRED = 8192
NGT = 32768
NCORES = 8
RPC = NPRED // NCORES  # rows per core = 1024
BLOCKS = RPC // P  # 8
G = 64  # group size for on-device segmented max
ST = 2048  # supertile columns (4 PSUM banks)
CHUNK = 8192  # yt streaming chunk columns
DVE_EIGHTHS = 8  # of each 8 supertiles, this many reduce on DVE (rest ACT+POOL)

GL_GROUPS = NGT // G  # 512
GN_GROUPS = NPRED // G  # 128

ALPHA = 100.0
MARGIN = 0.3
EPS = 1e-05

# set by test harness to capture a profile
TRACE = False
LAST_RESULTS = None


def _build_kernel():
    nc = bacc.Bacc("TRN2", debug=False, enable_asserts=False)

    xt = nc.dram_tensor("xt", [K, RPC], BF16, kind="ExternalInput").ap()
    yt = nc.dram_tensor("yt", [K, NGT], BF16, kind="ExternalInput").ap()
    pt = nc.dram_tensor("pt", [K, NPRED], BF16, kind="ExternalInput").ap()
    dmask = nc.dram_tensor("dmask", [P, P], F32, kind="ExternalInput").ap()
    gl = nc.dram_tensor("gl", [P, BLOCKS * GL_GROUPS], F32, kind="ExternalOutput").ap()
    gn = nc.dram_tensor("gn", [P, BLOCKS * GN_GROUPS], F32, kind="ExternalOutput").ap()

    n_chunks = NGT // CHUNK
    st_per_chunk = CHUNK // ST
    nxn_st = NPRED // ST
    st_groups = ST // G  # groups per supertile = 32

    with tile.TileContext(nc) as tc:
        with (
            tc.tile_pool(name="consts", bufs=1) as consts,
            tc.tile_pool(name="ytp", bufs=2) as ytp,
            tc.tile_pool(name="psum", bufs=2, space="PSUM") as psum,
            tc.tile_pool(name="copyb", bufs=3) as copyb,
            tc.tile_pool(name="treea", bufs=3) as treea,
            tc.tile_pool(name="treeb", bufs=3) as treeb,
            tc.tile_pool(name="acc", bufs=1) as accp,
        ):
            xt_s = consts.tile([K, RPC], BF16, tag="xt")
            nc.sync.dma_start(xt_s[:], xt)
            pt_s = consts.tile([K, NPRED], BF16, tag="pt")
            nc.sync.dma_start(pt_s[:], pt)
            dm_s = consts.tile([P, P], F32, tag="dm")
            nc.sync.dma_start(dm_s[:], dmask)

            glall = accp.tile([P, BLOCKS * GL_GROUPS], F32, tag="glall")
            gnall = accp.tile([P, BLOCKS * GN_GROUPS], F32, tag="gnall")
            nc.gpsimd.memset(glall[:], 0.0)
            nc.gpsimd.memset(gnall[:], 0.0)

            st_ctr = [0]

            def consume(ps, out_slice):
                """Segmented max: psum supertile [P, ST] -> out_slice [P, ST//G]."""
                use_dve = (st_ctr[0] % 8) < DVE_EIGHTHS
                st_ctr[0] += 1
                if use_dve:
                    nc.vector.tensor_reduce(
                        out=out_slice,
                        in_=ps.rearrange("p (g k) -> p g k", k=G),
                        axis=mybir.AxisListType.X,
                        op=mybir.AluOpType.max,
                    )
                    return
                cp = copyb.tile([P, ST], F32, tag="cp")
                nc.scalar.copy(out=cp[:], in_=ps)
                # grouped pairwise-max tree (ping-pong) down to width 1
                ta = treea.tile([P, ST // 2], F32, tag="ta")
                tb = treeb.tile([P, ST // 4], F32, tag="tb")
                w = G
                src = cp
                dsts = [ta, tb]
                d_i = 0
                while w > 1:
                    hw = w // 2
                    sv = src[:, : st_groups * w].rearrange("p (g w) -> p g w", w=w)
                    dst = dsts[d_i] if hw > 1 else None
                    if dst is not None:
                        dv = dst[:, : st_groups * hw].rearrange(
                            "p (g w) -> p g w", w=hw
                        )
                    else:
                        dv = out_slice.rearrange("p (g w) -> p g w", w=1)
                    nc.gpsimd.tensor_tensor(
                        out=dv, in0=sv[:, :, :hw], in1=sv[:, :, hw:],
                        op=mybir.AluOpType.max,
                    )
                    src = dst
                    d_i ^= 1
                    w = hw

            # ---- pred x gt ----
            for c in range(n_chunks):
                yt_s = ytp.tile([K, CHUNK], BF16, tag="yt")
                nc.sync.dma_start(yt_s[:], yt[:, c * CHUNK : (c + 1) * CHUNK])
                for r in range(BLOCKS):
                    for s in range(st_per_chunk):
                        ps = psum.tile([P, ST], F32, tag="ps")
                        for m in range(ST // 512):
                            nc.tensor.matmul(
                                out=ps[:, m * 512 : (m + 1) * 512],
                                lhsT=xt_s[:, r * P : (r + 1) * P],
                                rhs=yt_s[:, s * ST + m * 512 : s * ST + (m + 1) * 512],
                                start=True,
                                stop=True,
                            )
                        base = r * GL_GROUPS + c * (CHUNK // G) + s * st_groups
                        consume(ps[:], glall[:, base : base + st_groups])

            # ---- pred x pred ---- (pt is rolled per-core: own rows at cols [0, RPC))
            for r in range(BLOCKS):
                for s in range(nxn_st):
                    ps = psum.tile([P, ST], F32, tag="ps")
                    for m in range(ST // 512):
                        nc.tensor.matmul(
                            out=ps[:, m * 512 : (m + 1) * 512],
                            lhsT=xt_s[:, r * P : (r + 1) * P],
                            rhs=pt_s[:, s * ST + m * 512 : s * ST + (m + 1) * 512],
                            start=True,
                            stop=True,
                        )
                    if s == (r * P) // ST:
                        off = (r * P) % ST
                        nc.vector.tensor_add(
                            out=ps[:, off : off + P],
                            in0=ps[:, off : off + P],
                            in1=dm_s[:],
                        )
                    base = r * GN_GROUPS + s * st_groups
                    consume(ps[:], gnall[:, base : base + st_groups])

            nc.sync.dma_start(out=gl, in_=glall[:])
            nc.sync.dma_start(out=gn, in_=gnall[:])
    nc.compile()
    return nc


_NC_CACHE = None


def _get_nc():
    global _NC_CACHE
    if _NC_CACHE is None:
        _NC_CACHE = _build_kernel()
    return _NC_CACHE


def kernel(pred_feat, pred_decoder, input_data, gt_data):
    global LAST_RESULTS
    pred_feat = np.asarray(pred_feat, dtype=np.float32)
    gt_data = np.asarray(gt_data, dtype=np.float32)
    pred = np.ascontiguousarray(pred_feat[:, :3])
    pred_n = np.ascontiguousarray(pred_feat[:, 3:])
    gt_pts = np.ascontiguousarray(gt_data[:, :3])
    gt_nrm = np.ascontiguousarray(gt_data[:, 3:])

    import ml_dtypes

    bf = ml_dtypes.bfloat16

    def split_hi_lo(x):
        hi = x.astype(bf).astype(np.float32)
        lo = (x - hi).astype(bf).astype(np.float32)
        return hi, lo

    def rhs_rows(pts):
        """[K, n] moving-operand rows for target points pts (n, 3)."""
        hi, lo = split_hi_lo(pts)
        s = (pts.astype(np.float64) ** 2).sum(1).astype(np.float32)
        shi, slo = split_hi_lo(s)
        out = np.concatenate([hi.T, lo.T, hi.T, shi[None], slo[None]], 0)
        return np.ascontiguousarray(out.astype(bf))

    def lhs_rows(pts):
        """[K, n] stationary rows for query points pts (n, 3)."""
        hi, lo = split_hi_lo(pts)
        ones = np.ones((1, pts.shape[0]), np.float32)
        out = np.concatenate([2 * hi.T, 2 * hi.T, 2 * lo.T, -ones, -ones], 0)
        return np.ascontiguousarray(out.astype(bf))

    yt = rhs_rows(gt_pts)
    dmask = np.zeros((P, P), np.float32)
    np.fill_diagonal(dmask, -1e30)

    in_maps = []
    for k in range(NCORES):
        rolled = np.roll(pred, -k * RPC, axis=0)
        in_maps.append(
            {
                "xt": lhs_rows(pred[k * RPC : (k + 1) * RPC]),
                "yt": yt,
                "pt": rhs_rows(rolled),
                "dmask": dmask,
            }
        )

    nc = _get_nc()
    res = run_bass_kernel_spmd(
        nc, in_maps, core_ids=list(range(NCORES)), trace=TRACE
    )
    LAST_RESULTS = res

    # ---- assemble per-row group maxima ----
    GL = np.empty((NPRED, GL_GROUPS), np.float32)
    GN = np.empty((NPRED, GN_GROUPS), np.float32)
    for k in range(NCORES):
        glk = res.results[k]["gl"].reshape(P, BLOCKS, GL_GROUPS)
        GL[k * RPC : (k + 1) * RPC] = glk.transpose(1, 0, 2).reshape(RPC, GL_GROUPS)
        gnk = res.results[k]["gn"].reshape(P, BLOCKS, GN_GROUPS)
        GN[k * RPC : (k + 1) * RPC] = gnk.transpose(1, 0, 2).reshape(RPC, GN_GROUPS)

    rows = np.arange(NPRED)

    # ---- nearest gt point: resolve winning group of 64 on host ----
    gstar = np.argmax(GL, axis=1)
    cand = gstar[:, None] * G + np.arange(G)[None, :]  # (NPRED, G)
    diff = pred[:, None, :] - gt_pts[cand]  # (NPRED, G, 3)
    d2 = np.einsum("ijk,ijk->ij", diff, diff)
    loc = np.argmin(d2, axis=1)
    jstar = cand[rows, loc]

    closest = gt_pts[jstar]
    attraction = np.mean(((pred - closest) ** 2).astype(np.float64))

    # ---- normal alignment ----
    cn = gt_nrm[jstar]
    pn_norm = np.maximum(np.sqrt((pred_n**2).sum(1, keepdims=True)), EPS)
    cn_norm = np.maximum(np.sqrt((cn**2).sum(1, keepdims=True)), EPS)
    cos = ((pred_n / pn_norm) * (cn / cn_norm)).sum(1)
    norm_loss = np.mean((1.0 - cos).astype(np.float64))

    # ---- repulsion: min distance to other pred points ----
    x2 = (pred.astype(np.float64) ** 2).sum(1)
    local = rows % RPC
    gc = local // G  # contaminated (diagonal-containing) group, in rolled coords
    core = rows // RPC
    GN2 = GN.copy()
    GN2[rows, gc] = -np.inf
    m1 = x2 - GN2.max(axis=1)  # min d^2 over all non-contaminated groups
    # recompute the contaminated group exactly (excluding self)
    candn = (gc[:, None] * G + np.arange(G)[None, :] + core[:, None] * RPC) % NPRED
    diffn = pred[:, None, :] - pred[candn]
    d2n = np.einsum("ijk,ijk->ij", diffn, diffn)
    d2n[candn == rows[:, None]] = np.inf
    m2 = d2n.min(axis=1)
    min_d2 = np.minimum(m1, m2)
    min_dist = np.sqrt(np.maximum(min_d2, 0.0))
    pen = np.logaddexp(0.0, ALPHA * (MARGIN - min_dist))
    repulsion = np.mean(pen**2)

    loss = attraction + repulsion + 10.0 * norm_loss
    return np.float32(loss)



# revision 9
# speedup vs baseline: 1.0367x; 1.0367x over previous
"""Trainium2 Bass kernel for nn_CombinedCriterionAEImpulse (retrieval_knn).

Computes, on 8 NeuronCores, the heavy part of the loss:
  - q[i, j]      = 2*p_i . g_j - |g_j|^2  over the full (8192, 32768) pred x gt
    grid (row max of q  <=>  row min of squared distance), reduced on-device to
    one scalar per 1024-column group.
  - qself[i, j]  = 2*p_i . p_j - |p_j|^2  over (8192, 8192) pred x pred
    (rolled per-core so own rows sit in group 0), reduced the same way.
Rows (pred points) are sharded across the 8 cores.  The host resolves the
winning 1024-wide group per row with an exact f64 recompute (cheap numpy
GEMMs), gathers gt points/normals, and combines the scalar loss terms.

Draining the PSUM distance tiles is the bottleneck (42M f32/core), so it is
split across two engines working directly from PSUM:
  A: DVE tensor_reduce(max) -> exact group max            (~1.2us/group)
  E: ACT activation(Exp, scale=beta, bias=-beta*|p_i|^2) with accum_out
     -> sum_j exp(-beta * d_ij^2), a selection-only group statistic
     (~1.04us/group, zero DVE time)
E is used for 2/3 of the pred-x-gt groups (attraction/normal terms only need
the *winning group* to be right; the host recomputes distances exactly inside
it, and on the exp(-beta d^2) scale A-columns are made comparable on host).
The repulsion term needs accurate distance *values*, so all pred-x-pred
groups use A (exact maxes), keeping that path bit-identical to a plain
tensor_reduce kernel.

(Notes from probing this stack: tensor_tensor_reduce hangs on hardware (its
custom-DVE ucode table is not loaded under this runtime), dual-PSUM sources
are rejected (NCC_IBVF027), every classic TPB compute op is rejected on the
TRN2 GpSimd/Pool queue, and DVE tensor_reduce supports no 2x perf modes - so
ACT-with-accum is the only stock second drain engine.)
"""

import numpy as np

try:
    import concourse.bass as bass
except ImportError:  # pragma: no cover
    import sys

    sys.path.insert(0, "/opt/trn_rl_repo")
    import concourse.bass as bass

import concourse.mybir as mybir
import concourse.tile as tile
from concourse import bacc
from concourse.bass_utils import run_bass_kernel_spmd

P = 128
F32 = mybir.dt.float32
BF16 = mybir.dt.bfloat16
K = 11

NPRED = 8192
NGT = 32768
NCORES = 8
RPC = NPRED // NCORES  # rows per core = 1024
BLOCKS = RPC // P  # 8
ST = 1024  # group columns (2 PSUM banks); one result scalar per group
CHUNK = 4096  # yt streaming chunk columns (4 groups)

GL_G = NGT // ST  # 32 groups per row (pred x gt)
GN_G = NPRED // ST  # 8 groups per row (pred x pred)

BETA = 400.0

ALPHA = 100.0
MARGIN = 0.3
EPS = 1e-05

# set by test harness to capture a profile
TRACE = False
LAST_RESULTS = None


def _gl_schedule():
    """Path per pred-x-gt group in device emission order (c, r, s) ->
    dict[(r, g)] = True for E (exp-sum) else False for A (exact max)."""
    is_E = {}
    ctr = 0
    for c in range(NGT // CHUNK):
        for r in range(BLOCKS):
            for s in range(CHUNK // ST):
                is_E[(r, c * (CHUNK // ST) + s)] = (ctr % 3) != 2
                ctr += 1
    return is_E


IS_E = _gl_schedule()


def _build_kernel():
    nc = bacc.Bacc("TRN2", debug=False, enable_asserts=False)

    xt = nc.dram_tensor("xt", [K, RPC], BF16, kind="ExternalInput").ap()
    bx = nc.dram_tensor("bx", [P, BLOCKS], F32, kind="ExternalInput").ap()
    yt = nc.dram_tensor("yt", [K, NGT], BF16, kind="ExternalInput").ap()
    pt = nc.dram_tensor("pt", [K, NPRED], BF16, kind="ExternalInput").ap()
    gl = nc.dram_tensor("gl", [P, BLOCKS * GL_G], F32, kind="ExternalOutput").ap()
    gn = nc.dram_tensor("gn", [P, BLOCKS * GN_G], F32, kind="ExternalOutput").ap()

    n_chunks = NGT // CHUNK
    st_per_chunk = CHUNK // ST  # 4

    with tile.TileContext(nc) as tc:
        with (
            tc.tile_pool(name="consts", bufs=1) as consts,
            tc.tile_pool(name="ytp", bufs=2) as ytp,
            tc.tile_pool(name="psum", bufs=4, space="PSUM") as psum,
            tc.tile_pool(name="acc", bufs=1) as accp,
        ):
            xt_s = consts.tile([K, RPC], BF16, tag="xt")
            nc.sync.dma_start(xt_s[:], xt)
            bx_s = consts.tile([P, BLOCKS], F32, tag="bx")
            nc.sync.dma_start(bx_s[:], bx)
            pt_s = consts.tile([K, NPRED], BF16, tag="pt")
            nc.sync.dma_start(pt_s[:], pt)

            glres = accp.tile([P, BLOCKS * GL_G], F32, tag="glres")
            gnres = accp.tile([P, BLOCKS * GN_G], F32, tag="gnres")

            def fill(rhs_tile, r, s):
                """Two 512-col matmuls into a fresh [P, ST] psum group tile."""
                ps = psum.tile([P, ST], F32, tag="ps")
                for m in range(2):
                    nc.tensor.matmul(
                        out=ps[:, m * 512 : (m + 1) * 512],
                        lhsT=xt_s[:, r * P : (r + 1) * P],
                        rhs=rhs_tile[:, s * ST + m * 512 : s * ST + (m + 1) * 512],
                        start=True,
                        stop=True,
                    )
                return ps

            # ---- pred x gt ----
            for c in range(n_chunks):
                yt_s = ytp.tile([K, CHUNK], BF16, tag="yt")
                nc.sync.dma_start(yt_s[:], yt[:, c * CHUNK : (c + 1) * CHUNK])
                for r in range(BLOCKS):
                    for s in range(st_per_chunk):
                        ps = fill(yt_s, r, s)
                        g = c * st_per_chunk + s
                        out_slice = glres[:, r * GL_G + g : r * GL_G + g + 1]
                        if IS_E[(r, g)]:
                            nc.scalar.activation(
                                out=ps[:],
                                in_=ps[:],
                                func=mybir.ActivationFunctionType.Exp,
                                bias=bx_s[:, r : r + 1],
                                scale=BETA,
                                accum_out=out_slice,
                            )
                        else:
                            nc.vector.tensor_reduce(
                                out=out_slice,
                                in_=ps.rearrange("p (g k) -> p g k", k=ST),
                                axis=mybir.AxisListType.X,
                                op=mybir.AluOpType.max,
                            )

            # ---- pred x pred ---- (pt is rolled per-core: own rows in group 0)
            for r in range(BLOCKS):
                for s in range(GN_G):
                    ps = fill(pt_s, r, s)
                    nc.vector.tensor_reduce(
                        out=gnres[:, r * GN_G + s : r * GN_G + s + 1],
                        in_=ps.rearrange("p (g k) -> p g k", k=ST),
                        axis=mybir.AxisListType.X,
                        op=mybir.AluOpType.max,
                    )

            nc.sync.dma_start(out=gl, in_=glres[:])
            nc.sync.dma_start(out=gn, in_=gnres[:])
    nc.compile()
    return nc


_NC_CACHE = None


def _get_nc():
    global _NC_CACHE
    if _NC_CACHE is None:
        _NC_CACHE = _build_kernel()
    return _NC_CACHE


def kernel(pred_feat, pred_decoder, input_data, gt_data):
    global LAST_RESULTS
    pred_feat = np.asarray(pred_feat, dtype=np.float32)
    gt_data = np.asarray(gt_data, dtype=np.float32)
    pred = np.ascontiguousarray(pred_feat[:, :3])
    pred_n = np.ascontiguousarray(pred_feat[:, 3:])
    gt_pts = np.ascontiguousarray(gt_data[:, :3])
    gt_nrm = np.ascontiguousarray(gt_data[:, 3:])

    import ml_dtypes

    bf = ml_dtypes.bfloat16

    def split_hi_lo(x):
        hi = x.astype(bf).astype(np.float32)
        lo = (x - hi).astype(bf).astype(np.float32)
        return hi, lo

    def rhs_rows(pts):
        """[K, n] moving-operand rows for target points pts (n, 3)."""
        hi, lo = split_hi_lo(pts)
        s = (pts.astype(np.float64) ** 2).sum(1).astype(np.float32)
        shi, slo = split_hi_lo(s)
        out = np.concatenate([hi.T, lo.T, hi.T, shi[None], slo[None]], 0)
        return np.ascontiguousarray(out.astype(bf))

    def lhs_rows(pts):
        """[K, n] stationary rows for query points pts (n, 3)."""
        hi, lo = split_hi_lo(pts)
        ones = np.ones((1, pts.shape[0]), np.float32)
        out = np.concatenate([2 * hi.T, 2 * hi.T, 2 * lo.T, -ones, -ones], 0)
        return np.ascontiguousarray(out.astype(bf))

    yt = rhs_rows(gt_pts)
    x2f = (pred.astype(np.float64) ** 2).sum(1).astype(np.float32)

    in_maps = []
    for k in range(NCORES):
        rolled = np.roll(pred, -k * RPC, axis=0)
        bxk = -BETA * x2f[k * RPC : (k + 1) * RPC].reshape(BLOCKS, P).T
        in_maps.append(
            {
                "xt": lhs_rows(pred[k * RPC : (k + 1) * RPC]),
                "bx": np.ascontiguousarray(bxk),
                "yt": yt,
                "pt": rhs_rows(rolled),
            }
        )

    nc = _get_nc()
    res = run_bass_kernel_spmd(
        nc, in_maps, core_ids=list(range(NCORES)), trace=TRACE
    )
    LAST_RESULTS = res

    # ---- assemble per-row group results ----
    GL = np.empty((NPRED, GL_G), np.float32)
    GN = np.empty((NPRED, GN_G), np.float32)
    for k in range(NCORES):
        glk = res.results[k]["gl"].reshape(P, BLOCKS, GL_G)
        GL[k * RPC : (k + 1) * RPC] = glk.transpose(1, 0, 2).reshape(RPC, GL_G)
        gnk = res.results[k]["gn"].reshape(P, BLOCKS, GN_G)
        GN[k * RPC : (k + 1) * RPC] = gnk.transpose(1, 0, 2).reshape(RPC, GN_G)

    pred64 = pred.astype(np.float64)
    gt64 = gt_pts.astype(np.float64)
    x2 = (pred64**2).sum(1)

    # ---- nearest gt point: selection statistic on the exp(-beta d^2) scale,
    # then resolve the winning group exactly on host ----
    GLstat = np.empty((NPRED, GL_G), np.float64)
    e_cols = np.array([IS_E[(r, g)] for r in range(BLOCKS) for g in range(GL_G)])
    e_cols = e_cols.reshape(BLOCKS, GL_G)
    for r in range(BLOCKS):
        rows = (np.arange(NPRED) % RPC) // P == r
        for g in range(GL_G):
            col = GL[rows, g].astype(np.float64)
            if e_cols[r, g]:
                GLstat[rows, g] = col
            else:
                GLstat[rows, g] = np.exp(BETA * (col - x2[rows]))

    gstar = np.argmax(GLstat, axis=1)
    jstar = np.empty(NPRED, np.int64)
    for g in range(GL_G):
        m = gstar == g
        if not m.any():
            continue
        Y = gt64[g * ST : (g + 1) * ST]
        D = x2[m, None] + (Y**2).sum(1)[None, :] - 2.0 * (pred64[m] @ Y.T)
        jstar[m] = g * ST + D.argmin(1)

    attraction = np.mean((pred64 - gt64[jstar]) ** 2)

    # ---- normal alignment ----
    cn = gt_nrm[jstar]
    pn_norm = np.maximum(np.sqrt((pred_n**2).sum(1, keepdims=True)), EPS)
    cn_norm = np.maximum(np.sqrt((cn**2).sum(1, keepdims=True)), EPS)
    cos = ((pred_n / pn_norm) * (cn / cn_norm)).sum(1)
    norm_loss = np.mean((1.0 - cos).astype(np.float64))

    # ---- repulsion: min distance to other pred points ----
    # group 0 (rolled) contains self; recompute it exactly per core.
    m_other = GN[:, 1:].max(axis=1).astype(np.float64)
    min_d2_other = x2 - m_other
    exact0 = np.empty(NPRED, np.float64)
    li = np.arange(RPC)
    for k in range(NCORES):
        rows = slice(k * RPC, (k + 1) * RPC)
        cand = (k * RPC + np.arange(ST)) % NPRED
        Y = pred64[cand]
        D = x2[rows, None] + (Y**2).sum(1)[None, :] - 2.0 * (pred64[rows] @ Y.T)
        D[li, li] = np.inf  # self sits at position li in its core's candidates
        exact0[rows] = D.min(1)
    min_d2 = np.minimum(exact0, min_d2_other)
    min_dist = np.sqrt(np.maximum(min_d2, 0.0))
    pen = np.logaddexp(0.0, ALPHA * (MARGIN - min_dist))
    repulsion = np.mean(pen**2)

    loss = attraction + repulsion + 10.0 * norm_loss
    return np.float32(loss)


# revision 12
# speedup vs baseline: 1.2663x; 1.2214x over previous
"""Trainium2 Bass kernel for nn_CombinedCriterionAEImpulse (retrieval_knn).

Computes, on 8 NeuronCores, the heavy part of the loss:
  - q[i, j]      = 2*p_i . g_j - |g_j|^2  over the full (8192, 32768) pred x gt
    grid (row max of q  <=>  row min of squared distance), reduced on-device to
    one scalar per 1024-column group.
  - qself[i, j]  = 2*p_i . p_j - |p_j|^2  over (8192, 8192) pred x pred
    (rolled per-core so own rows sit in group 0), reduced the same way.
Rows (pred points) are sharded across the 8 cores.  The host resolves the
winning 1024-wide group per row with an exact f64 recompute (cheap numpy
GEMMs), gathers gt points/normals, and combines the scalar loss terms.

Draining the PSUM distance tiles is the bottleneck (42M f32/core), so it is
split across two engines working directly from PSUM:
  A: DVE tensor_reduce(max) -> exact group max            (~1.2us/group)
  E: ACT activation(Exp, scale=beta, bias=-beta*|p_i|^2) with accum_out
     -> sum_j exp(-beta * d_ij^2), a selection-only group statistic
     (~1.04us/group, zero DVE time)
E is used for 2/3 of the pred-x-gt groups (attraction/normal terms only need
the *winning group* to be right; the host recomputes distances exactly inside
it, and on the exp(-beta d^2) scale A-columns are made comparable on host).
The repulsion term needs accurate distance *values*, so all pred-x-pred
groups use A (exact maxes), keeping that path bit-identical to a plain
tensor_reduce kernel.

(Notes from probing this stack: tensor_tensor_reduce hangs on hardware (its
custom-DVE ucode table is not loaded under this runtime), dual-PSUM sources
are rejected (NCC_IBVF027), every classic TPB compute op is rejected on the
TRN2 GpSimd/Pool queue, and DVE tensor_reduce supports no 2x perf modes - so
ACT-with-accum is the only stock second drain engine.)
"""

import numpy as np

try:
    import concourse.bass as bass
except ImportError:  # pragma: no cover
    import sys

    sys.path.insert(0, "/opt/trn_rl_repo")
    import concourse.bass as bass

import concourse.mybir as mybir
import concourse.tile as tile
from concourse import bacc
from concourse.bass_utils import run_bass_kernel_spmd

P = 128
F32 = mybir.dt.float32
BF16 = mybir.dt.bfloat16
K = 11

NPRED = 8192
NGT = 32768
NCORES = 8
RPC = NPRED // NCORES  # rows per core = 1024
BLOCKS = RPC // P  # 8
ST = 1024  # group columns (2 PSUM banks); one result scalar per group
CHUNK = 4096  # yt streaming chunk columns (4 groups)

GL_G = NGT // ST  # 32 groups per row (pred x gt)
GN_G = NPRED // ST  # 8 groups per row (pred x pred)

BETA = 400.0

ALPHA = 100.0
MARGIN = 0.3
EPS = 1e-05

# set by test harness to capture a profile
TRACE = False
LAST_RESULTS = None


def _schedule():
    """Unified device task list.  GL tasks stay in (chunk, block, s) order for
    yt streaming; GN tasks are interleaved among them (after the first chunk)
    so the DVE drain they need is spread across the run instead of forming an
    all-DVE tail that throttles the PE.  Returns (tasks, is_E) where tasks is
    a list of ('gl'|'gn', r, g) and is_E maps GL (r, g) -> exp-sum path."""
    st_per_chunk = CHUNK // ST
    gl = [
        ("gl", r, c * st_per_chunk + s)
        for c in range(NGT // CHUNK)
        for r in range(BLOCKS)
        for s in range(st_per_chunk)
    ]
    gn = [("gn", r, s) for r in range(BLOCKS) for s in range(GN_G)]
    head = BLOCKS * st_per_chunk  # first chunk: GL only (pt DMA still landing)
    tasks = gl[:head]
    rest = gl[head:]
    debt = 0.0
    ratio = len(gn) / len(rest)
    gi = 0
    for t in rest:
        tasks.append(t)
        debt += ratio
        while debt >= 1.0 and gi < len(gn):
            tasks.append(gn[gi])
            gi += 1
            debt -= 1.0
    tasks.extend(gn[gi:])
    is_E = {}
    ctr = 0
    for kind, r, g in tasks:
        if kind == "gl":
            is_E[(r, g)] = (ctr % 3) != 2
            ctr += 1
    return tasks, is_E


TASKS, IS_E = _schedule()


def _build_kernel():
    nc = bacc.Bacc("TRN2", debug=False, enable_asserts=False)

    xt = nc.dram_tensor("xt", [K, RPC], BF16, kind="ExternalInput").ap()
    bx = nc.dram_tensor("bx", [P, BLOCKS], F32, kind="ExternalInput").ap()
    yt = nc.dram_tensor("yt", [K, NGT], BF16, kind="ExternalInput").ap()
    pt = nc.dram_tensor("pt", [K, NPRED], BF16, kind="ExternalInput").ap()
    gl = nc.dram_tensor("gl", [P, BLOCKS * GL_G], F32, kind="ExternalOutput").ap()
    gn = nc.dram_tensor("gn", [P, BLOCKS * GN_G], F32, kind="ExternalOutput").ap()

    n_chunks = NGT // CHUNK
    st_per_chunk = CHUNK // ST  # 4

    with tile.TileContext(nc) as tc:
        with (
            tc.tile_pool(name="consts", bufs=1) as consts,
            tc.tile_pool(name="ytp", bufs=2) as ytp,
            tc.tile_pool(name="psum", bufs=4, space="PSUM") as psum,
            tc.tile_pool(name="acc", bufs=1) as accp,
        ):
            xt_s = consts.tile([K, RPC], BF16, tag="xt")
            nc.sync.dma_start(xt_s[:], xt)
            bx_s = consts.tile([P, BLOCKS], F32, tag="bx")
            nc.sync.dma_start(bx_s[:], bx)
            yt_tiles = {}
            yt_tiles[0] = ytp.tile([K, CHUNK], BF16, tag="yt", name="yt0")
            nc.sync.dma_start(yt_tiles[0][:], yt[:, 0:CHUNK])
            pt_s = consts.tile([K, NPRED], BF16, tag="pt")
            nc.sync.dma_start(pt_s[:], pt)

            glres = accp.tile([P, BLOCKS * GL_G], F32, tag="glres")
            gnres = accp.tile([P, BLOCKS * GN_G], F32, tag="gnres")

            def fill(rhs_tile, r, s):
                """Two 512-col matmuls into a fresh [P, ST] psum group tile."""
                ps = psum.tile([P, ST], F32, tag="ps")
                for m in range(2):
                    nc.tensor.matmul(
                        out=ps[:, m * 512 : (m + 1) * 512],
                        lhsT=xt_s[:, r * P : (r + 1) * P],
                        rhs=rhs_tile[:, s * ST + m * 512 : s * ST + (m + 1) * 512],
                        start=True,
                        stop=True,
                    )
                return ps

            for kind, r, g in TASKS:
                if kind == "gl":
                    c = g // st_per_chunk
                    if c not in yt_tiles:
                        yt_tiles[c] = ytp.tile(
                            [K, CHUNK], BF16, tag="yt", name=f"yt{c}"
                        )
                        nc.sync.dma_start(
                            yt_tiles[c][:], yt[:, c * CHUNK : (c + 1) * CHUNK]
                        )
                    ps = fill(yt_tiles[c], r, g % st_per_chunk)
                    out_slice = glres[:, r * GL_G + g : r * GL_G + g + 1]
                    if IS_E[(r, g)]:
                        nc.scalar.activation(
                            out=ps[:],
                            in_=ps[:],
                            func=mybir.ActivationFunctionType.Exp,
                            bias=bx_s[:, r : r + 1],
                            scale=BETA,
                            accum_out=out_slice,
                        )
                    else:
                        nc.vector.tensor_reduce(
                            out=out_slice,
                            in_=ps.rearrange("p (g k) -> p g k", k=ST),
                            axis=mybir.AxisListType.X,
                            op=mybir.AluOpType.max,
                        )
                else:
                    ps = fill(pt_s, r, g)
                    nc.vector.tensor_reduce(
                        out=gnres[:, r * GN_G + g : r * GN_G + g + 1],
                        in_=ps.rearrange("p (g k) -> p g k", k=ST),
                        axis=mybir.AxisListType.X,
                        op=mybir.AluOpType.max,
                    )

            nc.sync.dma_start(out=gl, in_=glres[:])
            nc.sync.dma_start(out=gn, in_=gnres[:])
    nc.compile()
    return nc


_NC_CACHE = None


def _get_nc():
    global _NC_CACHE
    if _NC_CACHE is None:
        _NC_CACHE = _build_kernel()
    return _NC_CACHE


def kernel(pred_feat, pred_decoder, input_data, gt_data):
    global LAST_RESULTS
    pred_feat = np.asarray(pred_feat, dtype=np.float32)
    gt_data = np.asarray(gt_data, dtype=np.float32)
    pred = np.ascontiguousarray(pred_feat[:, :3])
    pred_n = np.ascontiguousarray(pred_feat[:, 3:])
    gt_pts = np.ascontiguousarray(gt_data[:, :3])
    gt_nrm = np.ascontiguousarray(gt_data[:, 3:])

    import ml_dtypes

    bf = ml_dtypes.bfloat16

    def split_hi_lo(x):
        hi = x.astype(bf).astype(np.float32)
        lo = (x - hi).astype(bf).astype(np.float32)
        return hi, lo

    def rhs_rows(pts):
        """[K, n] moving-operand rows for target points pts (n, 3)."""
        hi, lo = split_hi_lo(pts)
        s = (pts.astype(np.float64) ** 2).sum(1).astype(np.float32)
        shi, slo = split_hi_lo(s)
        out = np.concatenate([hi.T, lo.T, hi.T, shi[None], slo[None]], 0)
        return np.ascontiguousarray(out.astype(bf))

    def lhs_rows(pts):
        """[K, n] stationary rows for query points pts (n, 3)."""
        hi, lo = split_hi_lo(pts)
        ones = np.ones((1, pts.shape[0]), np.float32)
        out = np.concatenate([2 * hi.T, 2 * hi.T, 2 * lo.T, -ones, -ones], 0)
        return np.ascontiguousarray(out.astype(bf))

    yt = rhs_rows(gt_pts)
    x2f = (pred.astype(np.float64) ** 2).sum(1).astype(np.float32)

    in_maps = []
    for k in range(NCORES):
        rolled = np.roll(pred, -k * RPC, axis=0)
        bxk = -BETA * x2f[k * RPC : (k + 1) * RPC].reshape(BLOCKS, P).T
        in_maps.append(
            {
                "xt": lhs_rows(pred[k * RPC : (k + 1) * RPC]),
                "bx": np.ascontiguousarray(bxk),
                "yt": yt,
                "pt": rhs_rows(rolled),
            }
        )

    nc = _get_nc()
    res = run_bass_kernel_spmd(
        nc, in_maps, core_ids=list(range(NCORES)), trace=TRACE
    )
    LAST_RESULTS = res

    # ---- assemble per-row group results ----
    GL = np.empty((NPRED, GL_G), np.float32)
    GN = np.empty((NPRED, GN_G), np.float32)
    for k in range(NCORES):
        glk = res.results[k]["gl"].reshape(P, BLOCKS, GL_G)
        GL[k * RPC : (k + 1) * RPC] = glk.transpose(1, 0, 2).reshape(RPC, GL_G)
        gnk = res.results[k]["gn"].reshape(P, BLOCKS, GN_G)
        GN[k * RPC : (k + 1) * RPC] = gnk.transpose(1, 0, 2).reshape(RPC, GN_G)

    pred64 = pred.astype(np.float64)
    gt64 = gt_pts.astype(np.float64)
    x2 = (pred64**2).sum(1)

    # ---- nearest gt point: selection statistic on the exp(-beta d^2) scale,
    # then resolve the winning group exactly on host ----
    GLstat = np.empty((NPRED, GL_G), np.float64)
    e_cols = np.array([IS_E[(r, g)] for r in range(BLOCKS) for g in range(GL_G)])
    e_cols = e_cols.reshape(BLOCKS, GL_G)
    for r in range(BLOCKS):
        rows = (np.arange(NPRED) % RPC) // P == r
        for g in range(GL_G):
            col = GL[rows, g].astype(np.float64)
            if e_cols[r, g]:
                GLstat[rows, g] = col
            else:
                GLstat[rows, g] = np.exp(BETA * (col - x2[rows]))

    gstar = np.argmax(GLstat, axis=1)
    jstar = np.empty(NPRED, np.int64)
    for g in range(GL_G):
        m = gstar == g
        if not m.any():
            continue
        Y = gt64[g * ST : (g + 1) * ST]
        D = x2[m, None] + (Y**2).sum(1)[None, :] - 2.0 * (pred64[m] @ Y.T)
        jstar[m] = g * ST + D.argmin(1)

    attraction = np.mean((pred64 - gt64[jstar]) ** 2)

    # ---- normal alignment ----
    cn = gt_nrm[jstar]
    pn_norm = np.maximum(np.sqrt((pred_n**2).sum(1, keepdims=True)), EPS)
    cn_norm = np.maximum(np.sqrt((cn**2).sum(1, keepdims=True)), EPS)
    cos = ((pred_n / pn_norm) * (cn / cn_norm)).sum(1)
    norm_loss = np.mean((1.0 - cos).astype(np.float64))

    # ---- repulsion: min distance to other pred points ----
    # group 0 (rolled) contains self; recompute it exactly per core.
    m_other = GN[:, 1:].max(axis=1).astype(np.float64)
    min_d2_other = x2 - m_other
    exact0 = np.empty(NPRED, np.float64)
    li = np.arange(RPC)
    for k in range(NCORES):
        rows = slice(k * RPC, (k + 1) * RPC)
        cand = (k * RPC + np.arange(ST)) % NPRED
        Y = pred64[cand]
        D = x2[rows, None] + (Y**2).sum(1)[None, :] - 2.0 * (pred64[rows] @ Y.T)
        D[li, li] = np.inf  # self sits at position li in its core's candidates
        exact0[rows] = D.min(1)
    min_d2 = np.minimum(exact0, min_d2_other)
    min_dist = np.sqrt(np.maximum(min_d2, 0.0))
    pen = np.logaddexp(0.0, ALPHA * (MARGIN - min_dist))
    repulsion = np.mean(pen**2)

    loss = attraction + repulsion + 10.0 * norm_loss
    return np.float32(loss)
